# revision 50
# baseline (speedup 1.0000x reference)
"""Causal multi-head self-attention (B=4, T=2048, C=1024, H=16) on 8 TRN2 NeuronCores.

Sharding: core = b*2 + g  (b = batch 0..3, g = head-group 0..1 of 8 heads each).
Data parallel over batch; tensor parallel over heads (column-parallel W_attn,
row-parallel W_proj). Each core returns a partial (T, C) output; the host sums
the two partials per batch (the TP all-reduce happens in the unshard step).

Per-core device kernel (bf16 matmuls, f32 accumulation), per 512-wide q chunk:
  1. qT/kT projection with heads on partitions; head pairs share a 128-row tile
  2. v_aug projection in natural [t, c] layout with an all-ones column per head
     (the ones column turns the softmax denominator into row 64 of the y^T psum)
  3. attention in transposed [s, q] layout -- no transposes anywhere:
       S^T block = (kT block)^T @ qT chunk  (head-even rows 0:64 / head-odd rows
       64:128 of the PE array run concurrently: disjoint row groups)
       E = exp(S/8) on ScalarE (no max-subtraction: |scores|/8 < ~7)
       causal mask = precomputed multiplicative 0/1 tile on diagonal blocks
       y^T_aug accumulates v_aug^T @ E over s blocks in PSUM
     y/denominator are copied out of PSUM immediately (VectorE), denominators of
     all 8 heads take one batched reciprocal_approx_fast, the per-q reciprocal
     row is broadcast across partitions with a free-dim-step-0 SBUF->SBUF DMA,
     and one VectorE multiply normalizes.
  4. row-parallel output projection of the finished q chunk (overlaps the next
     chunk's attention).
"""

import numpy as np
import ml_dtypes

B, T, C, H = 4, 2048, 1024, 16
HS = C // H          # 64
NHL = 8              # local heads per core
KT = C // 128        # 8 contraction subtiles
NQC = T // 512       # 4 query chunks
NTB = T // 128       # 16 t-blocks
Bb16 = ml_dtypes.bfloat16

_CACHE = {}


def _build():
    import concourse.bass as bass
    import concourse.bacc as bacc
    import concourse.tile as tile
    import concourse.mybir as mybir

    BF = mybir.dt.bfloat16
    F32 = mybir.dt.float32
    AF = mybir.ActivationFunctionType

    nc = bacc.Bacc("TRN2", target_bir_lowering=False, debug=False, num_devices=8)
    xT = nc.dram_tensor("xT", [C, T], BF, kind="ExternalInput").ap()
    wqk = nc.dram_tensor("wqk", [C, 1024], BF, kind="ExternalInput").ap()
    wv = nc.dram_tensor("wv", [C, 520], BF, kind="ExternalInput").ap()
    wp = nc.dram_tensor("wp", [512, C], BF, kind="ExternalInput").ap()
    mask = nc.dram_tensor("mask", [128, 1280], BF, kind="ExternalInput").ap()
    out = nc.dram_tensor("out", [T, C], BF, kind="ExternalOutput").ap()

    with tile.TileContext(nc) as tc:
        with tc.tile_pool(name="persist", bufs=1) as persist, \
             tc.tile_pool(name="mm", bufs=2, space="PSUM") as mmpool, \
             tc.tile_pool(name="s", bufs=2, space="PSUM") as spool, \
             tc.tile_pool(name="av", bufs=2, space="PSUM") as avpool, \
             tc.tile_pool(name="e", bufs=4) as epool, \
             tc.tile_pool(name="nrm", bufs=2) as nrmpool, \
             tc.tile_pool(name="osb", bufs=3) as outpool:

            xT_sb = persist.tile([128, KT, T], BF, tag="xT")
            wqk_sb = persist.tile([128, KT, 1024], BF, tag="wqk")
            wv_sb = persist.tile([128, KT, 520], BF, tag="wv")
            wp_sb = persist.tile([128, 4, 1024], BF, tag="wp")
            mask_sb = persist.tile([128, 1280], BF, tag="mask")
            qk_sb = persist.tile([128, 8, T], BF, tag="qk")
            v_sb = persist.tile([128, NTB, 520], BF, tag="v")
            yT_sb = persist.tile([128, 4, T], BF, tag="yT")
            ones_sb = persist.tile([1, 64], BF, tag="ones")
            nc.vector.memset(ones_sb[:], 1.0)

            # load order: exactly what the first projection group needs, first
            for k in range(KT):
                nc.sync.dma_start(wqk_sb[:, k, 0:128], wqk[k * 128:(k + 1) * 128, 0:128])
                nc.sync.dma_start(xT_sb[:, k, 0:512], xT[k * 128:(k + 1) * 128, 0:512])
            for m in range(1, 8):
                for k in range(KT):
                    nc.sync.dma_start(wqk_sb[:, k, m * 128:(m + 1) * 128],
                                      wqk[k * 128:(k + 1) * 128, m * 128:(m + 1) * 128])
            for k in range(KT):
                nc.sync.dma_start(wv_sb[:, k, :], wv[k * 128:(k + 1) * 128, :])
            nc.sync.dma_start(mask_sb[:], mask[:])
            for k in range(KT):
                nc.sync.dma_start(xT_sb[:, k, 512:1024],
                                  xT[k * 128:(k + 1) * 128, 512:1024])
            for k in range(4):
                nc.sync.dma_start(wp_sb[:, k, :], wp[k * 128:(k + 1) * 128, :])
            for k in range(KT):
                nc.sync.dma_start(xT_sb[:, k, 1024:2048],
                                  xT[k * 128:(k + 1) * 128, 1024:2048])

            def qk_group(qc, m):
                q0 = qc * 512
                mm_ps = mmpool.tile([128, 512], F32, tag="mm")
                for k in range(KT):
                    nc.tensor.matmul(
                        mm_ps[:], wqk_sb[:, k, m * 128:(m + 1) * 128],
                        xT_sb[:, k, q0:q0 + 512],
                        start=(k == 0), stop=(k == KT - 1))
                nc.vector.tensor_copy(qk_sb[:, m, q0:q0 + 512], mm_ps[:])

            def v_group(j):
                jj = j * 128
                vps = mmpool.tile([128, 512], F32, tag="mm")
                for k in range(KT):
                    nc.tensor.matmul(
                        vps[:], xT_sb[:, k, jj:jj + 128],
                        wv_sb[:, k, 0:512],
                        start=(k == 0), stop=(k == KT - 1))
                nc.vector.tensor_copy(v_sb[:, j, 0:512], vps[:])
                vps2 = mmpool.tile([128, 8], F32, tag="mm")
                for k in range(KT):
                    nc.tensor.matmul(
                        vps2[:], xT_sb[:, k, jj:jj + 128],
                        wv_sb[:, k, 512:520],
                        start=(k == 0), stop=(k == KT - 1))
                nc.vector.tensor_copy(v_sb[:, j, 512:520], vps2[:])
                vones = v_sb[:, j, :].rearrange("p (h e) -> p h e", e=65)[:, :, 64]
                nc.vector.memset(vones, 1.0)

            def proj_groups(qc):
                return [lambda m=m: qk_group(qc, m) for m in range(8)] + \
                       [lambda j=j: v_group(j) for j in range(4 * qc, 4 * qc + 4)]

            def attention_pair(qc, hp, yraw_sb, den8_sb, fill=None):
                q0 = qc * 512
                hA, hB = 2 * hp, 2 * hp + 1
                avA = avpool.tile([65, 512], F32, tag="av", name=f"avA_{qc}_{hp}")
                avB = avpool.tile([65, 512], F32, tag="av", name=f"avB_{qc}_{hp}")
                # rounds: (j, psum offset, q offset, width) pairs; full-width
                # off-diagonal rounds first, then the two triangular diagonal
                # rounds with masked-out columns trimmed away
                rounds = []
                for jg in range(2 * qc):
                    rounds.append(([(2 * jg, 0, 0, 512), (2 * jg + 1, 512, 0, 512)],
                                   False))
                rounds.append(([(4 * qc, 0, 0, 512), (4 * qc + 1, 512, 128, 384)],
                               True))
                rounds.append(([(4 * qc + 2, 0, 256, 256), (4 * qc + 3, 256, 384, 128)],
                               True))
                for ri, (blocks, diag) in enumerate(rounds):
                    sA = spool.tile([128, 1024], F32, tag="s", name=f"sA_{qc}_{hp}_{ri}")
                    sB = spool.tile([128, 1024], F32, tag="s", name=f"sB_{qc}_{hp}_{ri}")
                    # the two j-blocks of one head's round target disjoint PE
                    # row groups (block 1 reads the partition-swapped copy), so
                    # they stream through the array concurrently
                    for j, off, qo, w in blocks:
                        for h, s_ps in ((hA, sA), (hB, sB)):
                            pb = (h % 2) * 64
                            nc.tensor.matmul(
                                s_ps[:, off:off + w],
                                qk_sb[pb:pb + 64, 4 + hp, j * 128:(j + 1) * 128],
                                qk_sb[pb:pb + 64, hp, q0 + qo:q0 + 512],
                                start=True, stop=True,
                                tile_position=(pb, 0))
                    wtot = blocks[-1][1] + blocks[-1][3]
                    for h, s_ps, av_ps in ((hA, sA, avA), (hB, sB, avB)):
                        e_sb = epool.tile([128, 1024], BF, tag="e")
                        nc.scalar.activation(e_sb[:, 0:wtot], s_ps[:, 0:wtot],
                                             AF.Exp, scale=0.125)
                        if diag:
                            moff = 0 if blocks[0][3] == 512 else 896
                            nc.vector.tensor_mul(
                                e_sb[:, 0:wtot], e_sb[:, 0:wtot],
                                mask_sb[:, moff:moff + wtot])
                        for j, off, qo, w in blocks:
                            nc.tensor.matmul(
                                av_ps[:, qo:512], v_sb[:, j, h * 65:h * 65 + 65],
                                e_sb[:, off:off + w],
                                start=(j == 0), stop=(j == 4 * qc + 3))
                    if fill is not None:
                        fill()
                # stash y and denominator; av psum slots free right away
                for h, av_ps in ((hA, avA), (hB, avB)):
                    with nc.allow_low_precision(reason="attention y bf16"):
                        nc.vector.tensor_copy(yraw_sb[:, h, :], av_ps[0:64, :])
                    p32 = (h % 4) * 32
                    nc.vector.tensor_copy(
                        den8_sb[p32:p32 + 1, h // 4, :], av_ps[64:65, :])

            def normalize_half(qc, half, yraw_sb, den8_sb):
                # heads 4*half .. 4*half+3 finished: reciprocal + scale them
                q0 = qc * 512
                r4_sb = nrmpool.tile([128, 512], F32, tag="r4", name=f"r4_{qc}_{half}")
                nc.vector.reciprocal_approx_fast(r4_sb[:], den8_sb[:, half, :])
                for h in range(4 * half, 4 * half + 4):
                    pb = (h % 2) * 64
                    p32 = (h % 4) * 32
                    r1_sb = nrmpool.tile([1, 512], BF, tag="r1")
                    with nc.allow_low_precision(reason="softmax denom bf16"):
                        nc.vector.tensor_copy(r1_sb[:], r4_sb[p32:p32 + 1, :])
                    # broadcast r across 64 partitions with a K=1 matmul into a
                    # spare slot of the S psum pool
                    bc_ps = spool.tile([128, 1024], F32, tag="s",
                                       name=f"bc_{qc}_{h}")[0:64, 0:512]
                    nc.tensor.matmul(bc_ps, ones_sb[:], r1_sb[:], start=True, stop=True)
                    with nc.allow_low_precision(reason="attention y bf16"):
                        nc.vector.tensor_mul(
                            yT_sb[pb:pb + 64, h // 2, q0:q0 + 512],
                            yraw_sb[:, h, :], bc_ps)

            def outproj_group(qc, tt):
                t0 = (qc * 4 + tt) * 128
                osb = outpool.tile([128, 1024], BF, tag="osb")
                for n in range(2):
                    ops = mmpool.tile([128, 512], F32, tag="mm")
                    for cp in range(4):
                        nc.tensor.matmul(
                            ops[:], yT_sb[:, cp, t0:t0 + 128],
                            wp_sb[:, cp, n * 512:(n + 1) * 512],
                            start=(cp == 0), stop=(cp == 3))
                    with nc.allow_low_precision(reason="bf16 partial output"):
                        nc.vector.tensor_copy(osb[:, n * 512:(n + 1) * 512], ops[:])
                nc.sync.dma_start(out[t0:t0 + 128, :], osb[:])

            def outproj_groups(qc):
                return [lambda tt=tt: outproj_group(qc, tt) for tt in range(4)]

            # software pipeline: projection/output matmul groups are emitted as
            # fillers inside the (ScalarE-bound) attention stretches. Each
            # filler is tagged with the q chunk whose attention requires it;
            # they are force-drained before that attention is emitted.
            from collections import deque
            for g in proj_groups(0):
                g()
            fillers = deque((1, g) for g in proj_groups(1))
            for qc in range(NQC):
                due = [g for tag, g in fillers if tag <= qc]
                fillers = deque((tag, g) for tag, g in fillers if tag > qc)
                for g in due:
                    g()
                yraw_sb = nrmpool.tile([64, NHL, 512], BF, tag="yraw", name=f"yraw{qc}")
                den8_sb = nrmpool.tile([128, 2, 512], F32, tag="den8", name=f"den8{qc}")
                nc.vector.memset(den8_sb[:], 1.0)
                nfill = [4, 3, 3, 4][qc]
                for hp in range(4):
                    attention_pair(qc, hp, yraw_sb, den8_sb)
                    if hp == 1:
                        normalize_half(qc, 0, yraw_sb, den8_sb)
                    npop = nfill if hp < 3 else nfill - 2
                    for _ in range(npop):
                        if fillers:
                            fillers.popleft()[1]()
                # keep two filler groups to cover the reciprocal chain latency
                for _ in range(2):
                    if fillers:
                        fillers.popleft()[1]()
                normalize_half(qc, 1, yraw_sb, den8_sb)
                fillers.extend((NQC, g) for g in outproj_groups(qc))
                if qc + 2 < NQC:
                    fillers.extend((qc + 2, g) for g in proj_groups(qc + 2))
            while fillers:
                fillers.popleft()[1]()
    nc.compile()
    return nc


def _get_nc():
    if "nc" not in _CACHE:
        _CACHE["nc"] = _build()
    return _CACHE["nc"]


def _host_prep(x, W_attn, W_proj):
    """Shard + lay out per-core inputs. Returns list of 8 in_maps."""
    x = np.asarray(x, dtype=np.float32)
    W_attn = np.asarray(W_attn, dtype=np.float32)
    W_proj = np.asarray(W_proj, dtype=np.float32)

    # triangular mask prefix: mask[s, i] = 1.0 if s <= i else 0
    s_idx = np.arange(128)[:, None]
    q_idx = np.arange(512)[None, :]
    tri = (s_idx <= q_idx).astype(np.float32)
    mask = np.ascontiguousarray(np.concatenate(
        [tri[:, :512], tri[:, :384], tri[:, :256], tri[:, :128]], axis=1
    )).astype(Bb16)

    xT_b = [np.ascontiguousarray(x[b].T).astype(Bb16) for b in range(B)]
    in_maps = []
    for core in range(8):
        b, g = core // 2, core % 2
        c0 = g * 512
        wqk_g = np.concatenate(
            [W_attn[:, c0:c0 + 512], W_attn[:, C + c0:C + c0 + 512]], axis=1
        ).astype(Bb16)
        vbase = W_attn[:, 2 * C + c0:2 * C + c0 + 512]
        wv_g = np.zeros((C, 520), dtype=np.float32)
        for h in range(NHL):
            wv_g[:, h * 65:h * 65 + 64] = vbase[:, h * 64:(h + 1) * 64]
        wp_g = np.ascontiguousarray(W_proj[c0:c0 + 512, :]).astype(Bb16)
        in_maps.append({
            "xT": xT_b[b],
            "wqk": np.ascontiguousarray(wqk_g),
            "wv": wv_g.astype(Bb16),
            "wp": wp_g,
            "mask": mask,
        })
    return in_maps


def kernel(x, W_attn, W_proj):
    from concourse import bass_utils

    nc = _get_nc()
    in_maps = _host_prep(x, W_attn, W_proj)
    res = bass_utils.run_bass_kernel_spmd(nc, in_maps, core_ids=list(range(8)))
    outs = [res.results[c]["out"] for c in range(8)]
    full = np.empty((B, T, C), dtype=np.float32)
    for b in range(B):
        full[b] = outs[2 * b].astype(np.float32) + outs[2 * b + 1].astype(np.float32)
    return full


# revision 51
# speedup vs baseline: 1.0741x; 1.0741x over previous
"""Causal multi-head self-attention (B=4, T=2048, C=1024, H=16) on 8 TRN2 NeuronCores.

Sharding: core = b*2 + g  (b = batch 0..3, g = head-group 0..1 of 8 heads each).
Data parallel over batch; tensor parallel over heads (column-parallel W_attn,
row-parallel W_proj). Each core returns a partial (T, C) output; the host sums
the two partials per batch (the TP all-reduce happens in the unshard step).

Per-core device kernel (bf16 matmuls, f32 accumulation), per 512-wide q chunk:
  1. qT/kT projection with heads on partitions; head pairs share a 128-row tile
  2. v_aug projection in natural [t, c] layout with an all-ones column per head
     (the ones column turns the softmax denominator into row 64 of the y^T psum)
  3. attention in transposed [s, q] layout -- no transposes anywhere:
       S^T block = (kT block)^T @ qT chunk  (head-even rows 0:64 / head-odd rows
       64:128 of the PE array run concurrently: disjoint row groups)
       E = exp(S/8) on ScalarE (no max-subtraction: |scores|/8 < ~7)
       causal mask = precomputed multiplicative 0/1 tile on diagonal blocks
       y^T_aug accumulates v_aug^T @ E over s blocks in PSUM
     y/denominator are copied out of PSUM immediately (VectorE), denominators of
     all 8 heads take one batched reciprocal_approx_fast, the per-q reciprocal
     row is broadcast across partitions with a free-dim-step-0 SBUF->SBUF DMA,
     and one VectorE multiply normalizes.
  4. row-parallel output projection of the finished q chunk (overlaps the next
     chunk's attention).
"""

import numpy as np
import ml_dtypes

B, T, C, H = 4, 2048, 1024, 16
HS = C // H          # 64
NHL = 8              # local heads per core
KT = C // 128        # 8 contraction subtiles
NQC = T // 512       # 4 query chunks
NTB = T // 128       # 16 t-blocks
Bb16 = ml_dtypes.bfloat16

_CACHE = {}


def _build():
    import concourse.bass as bass
    import concourse.bacc as bacc
    import concourse.tile as tile
    import concourse.mybir as mybir

    BF = mybir.dt.bfloat16
    F32 = mybir.dt.float32
    AF = mybir.ActivationFunctionType

    nc = bacc.Bacc("TRN2", target_bir_lowering=False, debug=False, num_devices=8)
    xT = nc.dram_tensor("xT", [C, T], BF, kind="ExternalInput").ap()
    wqk = nc.dram_tensor("wqk", [C, 1024], BF, kind="ExternalInput").ap()
    wv = nc.dram_tensor("wv", [C, 520], BF, kind="ExternalInput").ap()
    wp = nc.dram_tensor("wp", [512, C], BF, kind="ExternalInput").ap()
    mask = nc.dram_tensor("mask", [128, 1280], BF, kind="ExternalInput").ap()
    out = nc.dram_tensor("out", [T, C], BF, kind="ExternalOutput").ap()

    with tile.TileContext(nc) as tc:
        with tc.tile_pool(name="persist", bufs=1) as persist, \
             tc.tile_pool(name="mm", bufs=2, space="PSUM") as mmpool, \
             tc.tile_pool(name="s", bufs=2, space="PSUM") as spool, \
             tc.tile_pool(name="av", bufs=2, space="PSUM") as avpool, \
             tc.tile_pool(name="e", bufs=4) as epool, \
             tc.tile_pool(name="nrm", bufs=2) as nrmpool, \
             tc.tile_pool(name="osb", bufs=3) as outpool:

            xT_sb = persist.tile([128, KT, T], BF, tag="xT")
            wqk_sb = persist.tile([128, KT, 1024], BF, tag="wqk")
            wv_sb = persist.tile([128, KT, 520], BF, tag="wv")
            wp_sb = persist.tile([128, 4, 1024], BF, tag="wp")
            mask_sb = persist.tile([128, 1280], BF, tag="mask")
            qk_sb = persist.tile([128, 8, T], BF, tag="qk")
            v_sb = persist.tile([128, NTB, 520], BF, tag="v")
            yT_sb = persist.tile([128, 4, T], BF, tag="yT")
            ones_sb = persist.tile([1, 64], BF, tag="ones")
            nc.vector.memset(ones_sb[:], 1.0)

            # load order: exactly what the first projection chunk needs, first
            for k in range(KT):
                nc.sync.dma_start(wqk_sb[:, k, :], wqk[k * 128:(k + 1) * 128, :])
                nc.sync.dma_start(xT_sb[:, k, 0:512], xT[k * 128:(k + 1) * 128, 0:512])
            for k in range(KT):
                nc.sync.dma_start(wv_sb[:, k, :], wv[k * 128:(k + 1) * 128, :])
            nc.sync.dma_start(mask_sb[:], mask[:])
            for k in range(KT):
                nc.sync.dma_start(xT_sb[:, k, 512:1024],
                                  xT[k * 128:(k + 1) * 128, 512:1024])
            for k in range(4):
                nc.sync.dma_start(wp_sb[:, k, :], wp[k * 128:(k + 1) * 128, :])
            for k in range(KT):
                nc.sync.dma_start(xT_sb[:, k, 1024:2048],
                                  xT[k * 128:(k + 1) * 128, 1024:2048])

            def qk_group(qc, m):
                q0 = qc * 512
                mm_ps = mmpool.tile([128, 512], F32, tag="mm")
                for k in range(KT):
                    nc.tensor.matmul(
                        mm_ps[:], wqk_sb[:, k, m * 128:(m + 1) * 128],
                        xT_sb[:, k, q0:q0 + 512],
                        start=(k == 0), stop=(k == KT - 1))
                nc.vector.tensor_copy(qk_sb[:, m, q0:q0 + 512], mm_ps[:])

            def v_group(j):
                jj = j * 128
                vps = mmpool.tile([128, 512], F32, tag="mm")
                for k in range(KT):
                    nc.tensor.matmul(
                        vps[:], xT_sb[:, k, jj:jj + 128],
                        wv_sb[:, k, 0:512],
                        start=(k == 0), stop=(k == KT - 1))
                nc.vector.tensor_copy(v_sb[:, j, 0:512], vps[:])
                vps2 = mmpool.tile([128, 8], F32, tag="mm")
                for k in range(KT):
                    nc.tensor.matmul(
                        vps2[:], xT_sb[:, k, jj:jj + 128],
                        wv_sb[:, k, 512:520],
                        start=(k == 0), stop=(k == KT - 1))
                nc.vector.tensor_copy(v_sb[:, j, 512:520], vps2[:])
                vones = v_sb[:, j, :].rearrange("p (h e) -> p h e", e=65)[:, :, 64]
                nc.vector.memset(vones, 1.0)

            def proj_groups(qc):
                return [lambda m=m: qk_group(qc, m) for m in range(8)] + \
                       [lambda j=j: v_group(j) for j in range(4 * qc, 4 * qc + 4)]

            def attention_pair(qc, hp, yraw_sb, den8_sb, fill=None):
                q0 = qc * 512
                hA, hB = 2 * hp, 2 * hp + 1
                avA = avpool.tile([65, 512], F32, tag="av", name=f"avA_{qc}_{hp}")
                avB = avpool.tile([65, 512], F32, tag="av", name=f"avB_{qc}_{hp}")
                # rounds: (j, psum offset, q offset, width) pairs; full-width
                # off-diagonal rounds first, then the two triangular diagonal
                # rounds with masked-out columns trimmed away
                rounds = []
                for jg in range(2 * qc):
                    rounds.append(([(2 * jg, 0, 0, 512), (2 * jg + 1, 512, 0, 512)],
                                   False))
                rounds.append(([(4 * qc, 0, 0, 512), (4 * qc + 1, 512, 128, 384)],
                               True))
                rounds.append(([(4 * qc + 2, 0, 256, 256), (4 * qc + 3, 256, 384, 128)],
                               True))
                for ri, (blocks, diag) in enumerate(rounds):
                    sA = spool.tile([128, 1024], F32, tag="s", name=f"sA_{qc}_{hp}_{ri}")
                    sB = spool.tile([128, 1024], F32, tag="s", name=f"sB_{qc}_{hp}_{ri}")
                    # the two j-blocks of one head's round target disjoint PE
                    # row groups (block 1 reads the partition-swapped copy), so
                    # they stream through the array concurrently
                    for j, off, qo, w in blocks:
                        for h, s_ps in ((hA, sA), (hB, sB)):
                            pb = (h % 2) * 64
                            nc.tensor.matmul(
                                s_ps[:, off:off + w],
                                qk_sb[pb:pb + 64, 4 + hp, j * 128:(j + 1) * 128],
                                qk_sb[pb:pb + 64, hp, q0 + qo:q0 + 512],
                                start=True, stop=True,
                                tile_position=(pb, 0))
                    wtot = blocks[-1][1] + blocks[-1][3]
                    for h, s_ps, av_ps in ((hA, sA, avA), (hB, sB, avB)):
                        e_sb = epool.tile([128, 1024], BF, tag="e")
                        nc.scalar.activation(e_sb[:, 0:wtot], s_ps[:, 0:wtot],
                                             AF.Exp, scale=0.125)
                        if diag:
                            moff = 0 if blocks[0][3] == 512 else 896
                            nc.vector.tensor_mul(
                                e_sb[:, 0:wtot], e_sb[:, 0:wtot],
                                mask_sb[:, moff:moff + wtot])
                        for j, off, qo, w in blocks:
                            nc.tensor.matmul(
                                av_ps[:, qo:512], v_sb[:, j, h * 65:h * 65 + 65],
                                e_sb[:, off:off + w],
                                start=(j == 0), stop=(j == 4 * qc + 3))
                    if fill is not None:
                        fill()
                # stash y and denominator; av psum slots free right away
                for h, av_ps in ((hA, avA), (hB, avB)):
                    with nc.allow_low_precision(reason="attention y bf16"):
                        nc.vector.tensor_copy(yraw_sb[:, h, :], av_ps[0:64, :])
                    p32 = (h % 4) * 32
                    nc.vector.tensor_copy(
                        den8_sb[p32:p32 + 1, h // 4, :], av_ps[64:65, :])

            def normalize_half(qc, half, yraw_sb, den8_sb):
                # heads 4*half .. 4*half+3 finished: reciprocal + scale them
                q0 = qc * 512
                r4_sb = nrmpool.tile([128, 512], F32, tag="r4", name=f"r4_{qc}_{half}")
                nc.vector.reciprocal_approx_fast(r4_sb[:], den8_sb[:, half, :])
                for h in range(4 * half, 4 * half + 4):
                    pb = (h % 2) * 64
                    p32 = (h % 4) * 32
                    r1_sb = nrmpool.tile([1, 512], BF, tag="r1")
                    with nc.allow_low_precision(reason="softmax denom bf16"):
                        nc.vector.tensor_copy(r1_sb[:], r4_sb[p32:p32 + 1, :])
                    # broadcast r across 64 partitions with a K=1 matmul into a
                    # spare slot of the S psum pool
                    bc_ps = spool.tile([128, 1024], F32, tag="s",
                                       name=f"bc_{qc}_{h}")[0:64, 0:512]
                    nc.tensor.matmul(bc_ps, ones_sb[:], r1_sb[:], start=True, stop=True)
                    with nc.allow_low_precision(reason="attention y bf16"):
                        nc.vector.tensor_mul(
                            yT_sb[pb:pb + 64, h // 2, q0:q0 + 512],
                            yraw_sb[:, h, :], bc_ps)

            def outproj_group(qc, tt):
                t0 = (qc * 4 + tt) * 128
                osb = outpool.tile([128, 1024], BF, tag="osb")
                for n in range(2):
                    ops = mmpool.tile([128, 512], F32, tag="mm")
                    for cp in range(4):
                        nc.tensor.matmul(
                            ops[:], yT_sb[:, cp, t0:t0 + 128],
                            wp_sb[:, cp, n * 512:(n + 1) * 512],
                            start=(cp == 0), stop=(cp == 3))
                    with nc.allow_low_precision(reason="bf16 partial output"):
                        nc.vector.tensor_copy(osb[:, n * 512:(n + 1) * 512], ops[:])
                nc.sync.dma_start(out[t0:t0 + 128, :], osb[:])

            def outproj_groups(qc):
                return [lambda tt=tt: outproj_group(qc, tt) for tt in range(4)]

            # software pipeline: projection/output matmul groups are emitted as
            # fillers inside the (ScalarE-bound) attention stretches. Each
            # filler is tagged with the q chunk whose attention requires it;
            # they are force-drained before that attention is emitted.
            from collections import deque
            for g in proj_groups(0):
                g()
            fillers = deque((1, g) for g in proj_groups(1))
            for qc in range(NQC):
                due = [g for tag, g in fillers if tag <= qc]
                fillers = deque((tag, g) for tag, g in fillers if tag > qc)
                for g in due:
                    g()
                yraw_sb = nrmpool.tile([64, NHL, 512], BF, tag="yraw", name=f"yraw{qc}")
                den8_sb = nrmpool.tile([128, 2, 512], F32, tag="den8", name=f"den8{qc}")
                nc.vector.memset(den8_sb[:], 1.0)
                nfill = [4, 3, 3, 4][qc]
                for hp in range(4):
                    attention_pair(qc, hp, yraw_sb, den8_sb)
                    if hp == 1:
                        normalize_half(qc, 0, yraw_sb, den8_sb)
                    npop = nfill if hp < 3 else nfill - 2
                    for _ in range(npop):
                        if fillers:
                            fillers.popleft()[1]()
                # keep two filler groups to cover the reciprocal chain latency
                for _ in range(2):
                    if fillers:
                        fillers.popleft()[1]()
                normalize_half(qc, 1, yraw_sb, den8_sb)
                fillers.extend((NQC, g) for g in outproj_groups(qc))
                if qc + 2 < NQC:
                    fillers.extend((qc + 2, g) for g in proj_groups(qc + 2))
            while fillers:
                fillers.popleft()[1]()
    nc.compile()
    return nc


def _get_nc():
    if "nc" not in _CACHE:
        _CACHE["nc"] = _build()
    return _CACHE["nc"]


def _host_prep(x, W_attn, W_proj):
    """Shard + lay out per-core inputs. Returns list of 8 in_maps."""
    x = np.asarray(x, dtype=np.float32)
    W_attn = np.asarray(W_attn, dtype=np.float32)
    W_proj = np.asarray(W_proj, dtype=np.float32)

    # triangular mask prefix: mask[s, i] = 1.0 if s <= i else 0
    s_idx = np.arange(128)[:, None]
    q_idx = np.arange(512)[None, :]
    tri = (s_idx <= q_idx).astype(np.float32)
    mask = np.ascontiguousarray(np.concatenate(
        [tri[:, :512], tri[:, :384], tri[:, :256], tri[:, :128]], axis=1
    )).astype(Bb16)

    xT_b = [np.ascontiguousarray(x[b].T).astype(Bb16) for b in range(B)]
    in_maps = []
    for core in range(8):
        b, g = core // 2, core % 2
        c0 = g * 512
        wqk_g = np.concatenate(
            [W_attn[:, c0:c0 + 512], W_attn[:, C + c0:C + c0 + 512]], axis=1
        ).astype(Bb16)
        vbase = W_attn[:, 2 * C + c0:2 * C + c0 + 512]
        wv_g = np.zeros((C, 520), dtype=np.float32)
        for h in range(NHL):
            wv_g[:, h * 65:h * 65 + 64] = vbase[:, h * 64:(h + 1) * 64]
        wp_g = np.ascontiguousarray(W_proj[c0:c0 + 512, :]).astype(Bb16)
        in_maps.append({
            "xT": xT_b[b],
            "wqk": np.ascontiguousarray(wqk_g),
            "wv": wv_g.astype(Bb16),
            "wp": wp_g,
            "mask": mask,
        })
    return in_maps


def kernel(x, W_attn, W_proj):
    from concourse import bass_utils

    nc = _get_nc()
    in_maps = _host_prep(x, W_attn, W_proj)
    res = bass_utils.run_bass_kernel_spmd(nc, in_maps, core_ids=list(range(8)))
    outs = [res.results[c]["out"] for c in range(8)]
    full = np.empty((B, T, C), dtype=np.float32)
    for b in range(B):
        full[b] = outs[2 * b].astype(np.float32) + outs[2 * b + 1].astype(np.float32)
    return full


# revision 52
# speedup vs baseline: 1.1003x; 1.0244x over previous
"""Causal multi-head self-attention (B=4, T=2048, C=1024, H=16) on 8 TRN2 NeuronCores.

Sharding: core = b*2 + g  (b = batch 0..3, g = head-group 0..1 of 8 heads each).
Data parallel over batch; tensor parallel over heads (column-parallel W_attn,
row-parallel W_proj). Each core returns a partial (T, C) output; the host sums
the two partials per batch (the TP all-reduce happens in the unshard step).

Per-core device kernel (bf16 matmuls, f32 accumulation), per 512-wide q chunk:
  1. qT/kT projection with heads on partitions; head pairs share a 128-row tile
  2. v_aug projection in natural [t, c] layout with an all-ones column per head
     (the ones column turns the softmax denominator into row 64 of the y^T psum)
  3. attention in transposed [s, q] layout -- no transposes anywhere:
       S^T block = (kT block)^T @ qT chunk  (head-even rows 0:64 / head-odd rows
       64:128 of the PE array run concurrently: disjoint row groups)
       E = exp(S/8) on ScalarE (no max-subtraction: |scores|/8 < ~7)
       causal mask = precomputed multiplicative 0/1 tile on diagonal blocks
       y^T_aug accumulates v_aug^T @ E over s blocks in PSUM
     y/denominator are copied out of PSUM immediately (VectorE), denominators of
     all 8 heads take one batched reciprocal_approx_fast, the per-q reciprocal
     row is broadcast across partitions with a free-dim-step-0 SBUF->SBUF DMA,
     and one VectorE multiply normalizes.
  4. row-parallel output projection of the finished q chunk (overlaps the next
     chunk's attention).
"""

import numpy as np
import ml_dtypes

B, T, C, H = 4, 2048, 1024, 16
HS = C // H          # 64
NHL = 8              # local heads per core
KT = C // 128        # 8 contraction subtiles
NQC = T // 512       # 4 query chunks
NTB = T // 128       # 16 t-blocks
Bb16 = ml_dtypes.bfloat16

_CACHE = {}


def _build():
    import concourse.bass as bass
    import concourse.bacc as bacc
    import concourse.tile as tile
    import concourse.mybir as mybir

    BF = mybir.dt.bfloat16
    F32 = mybir.dt.float32
    AF = mybir.ActivationFunctionType

    nc = bacc.Bacc("TRN2", target_bir_lowering=False, debug=False, num_devices=8)
    xT = nc.dram_tensor("xT", [C, T], BF, kind="ExternalInput").ap()
    wqk = nc.dram_tensor("wqk", [C, 1024], BF, kind="ExternalInput").ap()
    wv = nc.dram_tensor("wv", [C, 520], BF, kind="ExternalInput").ap()
    wp = nc.dram_tensor("wp", [512, C], BF, kind="ExternalInput").ap()
    mask = nc.dram_tensor("mask", [128, 1280], BF, kind="ExternalInput").ap()
    out = nc.dram_tensor("out", [T, C], BF, kind="ExternalOutput").ap()

    with tile.TileContext(nc) as tc:
        with tc.tile_pool(name="persist", bufs=1) as persist, \
             tc.tile_pool(name="mm", bufs=2, space="PSUM") as mmpool, \
             tc.tile_pool(name="s", bufs=2, space="PSUM") as spool, \
             tc.tile_pool(name="av", bufs=2, space="PSUM") as avpool, \
             tc.tile_pool(name="e", bufs=4) as epool, \
             tc.tile_pool(name="nrm", bufs=2) as nrmpool, \
             tc.tile_pool(name="osb", bufs=3) as outpool:

            xT_sb = persist.tile([128, KT, T], BF, tag="xT")
            wqk_sb = persist.tile([128, KT, 1024], BF, tag="wqk")
            wv_sb = persist.tile([128, KT, 520], BF, tag="wv")
            wp_sb = persist.tile([128, 4, 1024], BF, tag="wp")
            mask_sb = persist.tile([128, 1280], BF, tag="mask")
            qk_sb = persist.tile([128, 8, T], BF, tag="qk")
            v_sb = persist.tile([128, NTB, 520], BF, tag="v")
            yT_sb = persist.tile([128, 4, T], BF, tag="yT")
            ones_sb = persist.tile([1, 64], BF, tag="ones")
            nc.vector.memset(ones_sb[:], 1.0)

            # load order: exactly what the first projection chunk needs, first
            for k in range(KT):
                nc.sync.dma_start(wqk_sb[:, k, :], wqk[k * 128:(k + 1) * 128, :])
                nc.sync.dma_start(xT_sb[:, k, 0:512], xT[k * 128:(k + 1) * 128, 0:512])
            for k in range(KT):
                nc.sync.dma_start(wv_sb[:, k, :], wv[k * 128:(k + 1) * 128, :])
            nc.sync.dma_start(mask_sb[:], mask[:])
            for k in range(KT):
                nc.sync.dma_start(xT_sb[:, k, 512:1024],
                                  xT[k * 128:(k + 1) * 128, 512:1024])
            for k in range(4):
                nc.sync.dma_start(wp_sb[:, k, :], wp[k * 128:(k + 1) * 128, :])
            for k in range(KT):
                nc.sync.dma_start(xT_sb[:, k, 1024:2048],
                                  xT[k * 128:(k + 1) * 128, 1024:2048])

            def qk_group(qc, m):
                q0 = qc * 512
                mm_ps = mmpool.tile([128, 512], F32, tag="mm")
                for k in range(KT):
                    nc.tensor.matmul(
                        mm_ps[:], wqk_sb[:, k, m * 128:(m + 1) * 128],
                        xT_sb[:, k, q0:q0 + 512],
                        start=(k == 0), stop=(k == KT - 1))
                nc.vector.tensor_copy(qk_sb[:, m, q0:q0 + 512], mm_ps[:])

            def v_group(j):
                jj = j * 128
                vps = mmpool.tile([128, 512], F32, tag="mm")
                for k in range(KT):
                    nc.tensor.matmul(
                        vps[:], xT_sb[:, k, jj:jj + 128],
                        wv_sb[:, k, 0:512],
                        start=(k == 0), stop=(k == KT - 1))
                nc.vector.tensor_copy(v_sb[:, j, 0:512], vps[:])
                vps2 = mmpool.tile([128, 8], F32, tag="mm")
                for k in range(KT):
                    nc.tensor.matmul(
                        vps2[:], xT_sb[:, k, jj:jj + 128],
                        wv_sb[:, k, 512:520],
                        start=(k == 0), stop=(k == KT - 1))
                nc.vector.tensor_copy(v_sb[:, j, 512:520], vps2[:])
                vones = v_sb[:, j, :].rearrange("p (h e) -> p h e", e=65)[:, :, 64]
                nc.vector.memset(vones, 1.0)

            def proj_groups(qc):
                return [lambda m=m: qk_group(qc, m) for m in range(8)] + \
                       [lambda j=j: v_group(j) for j in range(4 * qc, 4 * qc + 4)]

            def attention_pair(qc, hp, yraw_sb, den8_sb, fill=None):
                q0 = qc * 512
                hA, hB = 2 * hp, 2 * hp + 1
                avA = avpool.tile([65, 512], F32, tag="av", name=f"avA_{qc}_{hp}")
                avB = avpool.tile([65, 512], F32, tag="av", name=f"avB_{qc}_{hp}")
                # rounds: (j, psum offset, q offset, width) pairs; full-width
                # off-diagonal rounds first, then the two triangular diagonal
                # rounds with masked-out columns trimmed away
                rounds = []
                for jg in range(2 * qc):
                    rounds.append(([(2 * jg, 0, 0, 512), (2 * jg + 1, 512, 0, 512)],
                                   False))
                rounds.append(([(4 * qc, 0, 0, 512), (4 * qc + 1, 512, 128, 384)],
                               True))
                rounds.append(([(4 * qc + 2, 0, 256, 256), (4 * qc + 3, 256, 384, 128)],
                               True))
                for ri, (blocks, diag) in enumerate(rounds):
                    sA = spool.tile([128, 1024], F32, tag="s", name=f"sA_{qc}_{hp}_{ri}")
                    sB = spool.tile([128, 1024], F32, tag="s", name=f"sB_{qc}_{hp}_{ri}")
                    # the two j-blocks of one head's round target disjoint PE
                    # row groups (block 1 reads the partition-swapped copy), so
                    # they stream through the array concurrently
                    for j, off, qo, w in blocks:
                        for h, s_ps in ((hA, sA), (hB, sB)):
                            pb = (h % 2) * 64
                            nc.tensor.matmul(
                                s_ps[:, off:off + w],
                                qk_sb[pb:pb + 64, 4 + hp, j * 128:(j + 1) * 128],
                                qk_sb[pb:pb + 64, hp, q0 + qo:q0 + 512],
                                start=True, stop=True,
                                tile_position=(pb, 0))
                    wtot = blocks[-1][1] + blocks[-1][3]
                    for h, s_ps, av_ps in ((hA, sA, avA), (hB, sB, avB)):
                        e_sb = epool.tile([128, 1024], BF, tag="e")
                        nc.scalar.activation(e_sb[:, 0:wtot], s_ps[:, 0:wtot],
                                             AF.Exp, scale=0.125)
                        if diag:
                            moff = 0 if blocks[0][3] == 512 else 896
                            nc.vector.tensor_mul(
                                e_sb[:, 0:wtot], e_sb[:, 0:wtot],
                                mask_sb[:, moff:moff + wtot])
                        for j, off, qo, w in blocks:
                            nc.tensor.matmul(
                                av_ps[:, qo:512], v_sb[:, j, h * 65:h * 65 + 65],
                                e_sb[:, off:off + w],
                                start=(j == 0), stop=(j == 4 * qc + 3))
                    if fill is not None:
                        fill()
                # stash y and denominator; av psum slots free right away
                for h, av_ps in ((hA, avA), (hB, avB)):
                    with nc.allow_low_precision(reason="attention y bf16"):
                        nc.vector.tensor_copy(yraw_sb[:, h, :], av_ps[0:64, :])
                    p32 = (h % 4) * 32
                    nc.vector.tensor_copy(
                        den8_sb[p32:p32 + 1, h // 4, :], av_ps[64:65, :])

            def normalize_half(qc, half, yraw_sb, den8_sb):
                # heads 4*half .. 4*half+3 finished: reciprocal + scale them
                q0 = qc * 512
                r4_sb = nrmpool.tile([128, 512], F32, tag="r4", name=f"r4_{qc}_{half}")
                nc.vector.reciprocal_approx_fast(r4_sb[:], den8_sb[:, half, :])
                for h in range(4 * half, 4 * half + 4):
                    pb = (h % 2) * 64
                    p32 = (h % 4) * 32
                    r1_sb = nrmpool.tile([1, 512], BF, tag="r1")
                    with nc.allow_low_precision(reason="softmax denom bf16"):
                        nc.vector.tensor_copy(r1_sb[:], r4_sb[p32:p32 + 1, :])
                    # broadcast r across 64 partitions with a K=1 matmul into a
                    # spare slot of the S psum pool
                    bc_ps = spool.tile([128, 1024], F32, tag="s",
                                       name=f"bc_{qc}_{h}")[0:64, 0:512]
                    nc.tensor.matmul(bc_ps, ones_sb[:], r1_sb[:], start=True, stop=True)
                    with nc.allow_low_precision(reason="attention y bf16"):
                        nc.vector.tensor_mul(
                            yT_sb[pb:pb + 64, h // 2, q0:q0 + 512],
                            yraw_sb[:, h, :], bc_ps)

            def outproj_group(qc, tt):
                t0 = (qc * 4 + tt) * 128
                osb = outpool.tile([128, 1024], BF, tag="osb")
                for n in range(2):
                    ops = mmpool.tile([128, 512], F32, tag="mm")
                    for cp in range(4):
                        nc.tensor.matmul(
                            ops[:], yT_sb[:, cp, t0:t0 + 128],
                            wp_sb[:, cp, n * 512:(n + 1) * 512],
                            start=(cp == 0), stop=(cp == 3))
                    with nc.allow_low_precision(reason="bf16 partial output"):
                        nc.vector.tensor_copy(osb[:, n * 512:(n + 1) * 512], ops[:])
                nc.sync.dma_start(out[t0:t0 + 128, :], osb[:])

            def outproj_groups(qc):
                return [lambda tt=tt: outproj_group(qc, tt) for tt in range(4)]

            def boot_qk_proj():
                # chunk-0 qT/kT projection with k as the OUTER loop: all 8
                # column-slot accumulators live at once (attention pools are
                # idle at kernel start, so all 8 PSUM banks are free) and the
                # first matmuls issue after only the first k-slice of DMA
                ps = []
                for m in range(8):
                    if m < 2:
                        ps.append(mmpool.tile([128, 512], F32, tag="mm",
                                              name=f"boot{m}"))
                    elif m < 6:
                        if m % 2 == 0:
                            st = spool.tile([128, 1024], F32, tag="s",
                                            name=f"boot{m}")
                        ps.append(st[:, (m % 2) * 512:(m % 2) * 512 + 512])
                    else:
                        ps.append(avpool.tile([128, 512], F32, tag="av",
                                              name=f"boot{m}"))
                for k in range(KT):
                    for m in range(8):
                        nc.tensor.matmul(
                            ps[m], wqk_sb[:, k, m * 128:(m + 1) * 128],
                            xT_sb[:, k, 0:512],
                            start=(k == 0), stop=(k == KT - 1))
                for m in range(8):
                    nc.vector.tensor_copy(qk_sb[:, m, 0:512], ps[m])

            # software pipeline: projection/output matmul groups are emitted as
            # fillers inside the (ScalarE-bound) attention stretches. Each
            # filler is tagged with the q chunk whose attention requires it;
            # they are force-drained before that attention is emitted.
            from collections import deque
            boot_qk_proj()
            for g in proj_groups(0)[8:]:
                g()
            fillers = deque((1, g) for g in proj_groups(1))
            for qc in range(NQC):
                due = [g for tag, g in fillers if tag <= qc]
                fillers = deque((tag, g) for tag, g in fillers if tag > qc)
                for g in due:
                    g()
                yraw_sb = nrmpool.tile([64, NHL, 512], BF, tag="yraw", name=f"yraw{qc}")
                den8_sb = nrmpool.tile([128, 2, 512], F32, tag="den8", name=f"den8{qc}")
                nc.vector.memset(den8_sb[:], 1.0)
                nfill = [4, 3, 3, 4][qc]
                for hp in range(4):
                    attention_pair(qc, hp, yraw_sb, den8_sb)
                    if hp == 1:
                        normalize_half(qc, 0, yraw_sb, den8_sb)
                    npop = nfill if hp < 3 else nfill - 2
                    for _ in range(npop):
                        if fillers:
                            fillers.popleft()[1]()
                # keep two filler groups to cover the reciprocal chain latency
                for _ in range(2):
                    if fillers:
                        fillers.popleft()[1]()
                normalize_half(qc, 1, yraw_sb, den8_sb)
                fillers.extend((NQC, g) for g in outproj_groups(qc))
                if qc + 2 < NQC:
                    fillers.extend((qc + 2, g) for g in proj_groups(qc + 2))
            while fillers:
                fillers.popleft()[1]()
    nc.compile()
    return nc


def _get_nc():
    if "nc" not in _CACHE:
        _CACHE["nc"] = _build()
    return _CACHE["nc"]


def _host_prep(x, W_attn, W_proj):
    """Shard + lay out per-core inputs. Returns list of 8 in_maps."""
    x = np.asarray(x, dtype=np.float32)
    W_attn = np.asarray(W_attn, dtype=np.float32)
    W_proj = np.asarray(W_proj, dtype=np.float32)

    # triangular mask prefix: mask[s, i] = 1.0 if s <= i else 0
    s_idx = np.arange(128)[:, None]
    q_idx = np.arange(512)[None, :]
    tri = (s_idx <= q_idx).astype(np.float32)
    mask = np.ascontiguousarray(np.concatenate(
        [tri[:, :512], tri[:, :384], tri[:, :256], tri[:, :128]], axis=1
    )).astype(Bb16)

    xT_b = [np.ascontiguousarray(x[b].T).astype(Bb16) for b in range(B)]
    in_maps = []
    for core in range(8):
        b, g = core // 2, core % 2
        c0 = g * 512
        wqk_g = np.concatenate(
            [W_attn[:, c0:c0 + 512], W_attn[:, C + c0:C + c0 + 512]], axis=1
        ).astype(Bb16)
        vbase = W_attn[:, 2 * C + c0:2 * C + c0 + 512]
        wv_g = np.zeros((C, 520), dtype=np.float32)
        for h in range(NHL):
            wv_g[:, h * 65:h * 65 + 64] = vbase[:, h * 64:(h + 1) * 64]
        wp_g = np.ascontiguousarray(W_proj[c0:c0 + 512, :]).astype(Bb16)
        in_maps.append({
            "xT": xT_b[b],
            "wqk": np.ascontiguousarray(wqk_g),
            "wv": wv_g.astype(Bb16),
            "wp": wp_g,
            "mask": mask,
        })
    return in_maps


def kernel(x, W_attn, W_proj):
    from concourse import bass_utils

    nc = _get_nc()
    in_maps = _host_prep(x, W_attn, W_proj)
    res = bass_utils.run_bass_kernel_spmd(nc, in_maps, core_ids=list(range(8)))
    outs = [res.results[c]["out"] for c in range(8)]
    full = np.empty((B, T, C), dtype=np.float32)
    for b in range(B):
        full[b] = outs[2 * b].astype(np.float32) + outs[2 * b + 1].astype(np.float32)
    return full


# revision 53
# speedup vs baseline: 1.1851x; 1.0770x over previous
"""Causal multi-head self-attention (B=4, T=2048, C=1024, H=16) on 8 TRN2 NeuronCores.

Sharding: core = b*2 + g  (b = batch 0..3, g = head-group 0..1 of 8 heads each).
Data parallel over batch; tensor parallel over heads (column-parallel W_attn,
row-parallel W_proj). Each core returns a partial (T, C) output; the host sums
the two partials per batch (the TP all-reduce happens in the unshard step).

Per-core device kernel (bf16 matmuls, f32 accumulation), per 512-wide q chunk:
  1. qT/kT projection with heads on partitions; head pairs share a 128-row tile
  2. v_aug projection in natural [t, c] layout with an all-ones column per head
     (the ones column turns the softmax denominator into row 64 of the y^T psum)
  3. attention in transposed [s, q] layout -- no transposes anywhere:
       S^T block = (kT block)^T @ qT chunk  (head-even rows 0:64 / head-odd rows
       64:128 of the PE array run concurrently: disjoint row groups)
       E = exp(S/8) on ScalarE (no max-subtraction: |scores|/8 < ~7)
       causal mask = precomputed multiplicative 0/1 tile on diagonal blocks
       y^T_aug accumulates v_aug^T @ E over s blocks in PSUM
     y/denominator are copied out of PSUM immediately (VectorE), denominators of
     all 8 heads take one batched reciprocal_approx_fast, the per-q reciprocal
     row is broadcast across partitions with a free-dim-step-0 SBUF->SBUF DMA,
     and one VectorE multiply normalizes.
  4. row-parallel output projection of the finished q chunk (overlaps the next
     chunk's attention).
"""

import numpy as np
import ml_dtypes

B, T, C, H = 4, 2048, 1024, 16
HS = C // H          # 64
NHL = 8              # local heads per core
KT = C // 128        # 8 contraction subtiles
NQC = T // 512       # 4 query chunks
NTB = T // 128       # 16 t-blocks
Bb16 = ml_dtypes.bfloat16

_CACHE = {}


def _build():
    import concourse.bass as bass
    import concourse.bacc as bacc
    import concourse.tile as tile
    import concourse.mybir as mybir

    BF = mybir.dt.bfloat16
    F32 = mybir.dt.float32
    AF = mybir.ActivationFunctionType

    nc = bacc.Bacc("TRN2", target_bir_lowering=False, debug=False, num_devices=8)
    xT = nc.dram_tensor("xT", [C, T], BF, kind="ExternalInput").ap()
    wqk = nc.dram_tensor("wqk", [C, 1024], BF, kind="ExternalInput").ap()
    wv = nc.dram_tensor("wv", [C, 520], BF, kind="ExternalInput").ap()
    wp = nc.dram_tensor("wp", [512, C], BF, kind="ExternalInput").ap()
    mask = nc.dram_tensor("mask", [128, 1280], BF, kind="ExternalInput").ap()
    out = nc.dram_tensor("out", [T, C], BF, kind="ExternalOutput").ap()

    with tile.TileContext(nc) as tc:
        with tc.tile_pool(name="persist", bufs=1) as persist, \
             tc.tile_pool(name="mm", bufs=2, space="PSUM") as mmpool, \
             tc.tile_pool(name="s", bufs=2, space="PSUM") as spool, \
             tc.tile_pool(name="av", bufs=2, space="PSUM") as avpool, \
             tc.tile_pool(name="e", bufs=4) as epool, \
             tc.tile_pool(name="nrm", bufs=2) as nrmpool, \
             tc.tile_pool(name="osb", bufs=3) as outpool:

            xT_sb = persist.tile([128, KT, T], BF, tag="xT")
            wqk_sb = persist.tile([128, KT, 1024], BF, tag="wqk")
            wv_sb = persist.tile([128, KT, 520], BF, tag="wv")
            wp_sb = persist.tile([128, 4, 1024], BF, tag="wp")
            mask_sb = persist.tile([128, 1280], BF, tag="mask")
            qk_sb = persist.tile([128, 8, T], BF, tag="qk")
            v_sb = persist.tile([128, NTB, 520], BF, tag="v")
            yT_sb = persist.tile([128, 4, T], BF, tag="yT")
            ones_sb = persist.tile([1, 64], BF, tag="ones")
            nc.vector.memset(ones_sb[:], 1.0)

            # load order: exactly what the first projection chunk needs, first
            for k in range(KT):
                nc.sync.dma_start(wqk_sb[:, k, :], wqk[k * 128:(k + 1) * 128, :])
                nc.sync.dma_start(xT_sb[:, k, 0:512], xT[k * 128:(k + 1) * 128, 0:512])
            for k in range(KT):
                nc.sync.dma_start(wv_sb[:, k, :], wv[k * 128:(k + 1) * 128, :])
            nc.sync.dma_start(mask_sb[:], mask[:])
            for k in range(KT):
                nc.sync.dma_start(xT_sb[:, k, 512:1024],
                                  xT[k * 128:(k + 1) * 128, 512:1024])
            for k in range(4):
                nc.sync.dma_start(wp_sb[:, k, :], wp[k * 128:(k + 1) * 128, :])
            for k in range(KT):
                nc.sync.dma_start(xT_sb[:, k, 1024:2048],
                                  xT[k * 128:(k + 1) * 128, 1024:2048])

            def qk_group(qc, m):
                q0 = qc * 512
                mm_ps = mmpool.tile([128, 512], F32, tag="mm")
                for k in range(KT):
                    nc.tensor.matmul(
                        mm_ps[:], wqk_sb[:, k, m * 128:(m + 1) * 128],
                        xT_sb[:, k, q0:q0 + 512],
                        start=(k == 0), stop=(k == KT - 1))
                nc.vector.tensor_copy(qk_sb[:, m, q0:q0 + 512], mm_ps[:])

            def v_group(j):
                jj = j * 128
                vps = mmpool.tile([128, 512], F32, tag="mm")
                for k in range(KT):
                    nc.tensor.matmul(
                        vps[:], xT_sb[:, k, jj:jj + 128],
                        wv_sb[:, k, 0:512],
                        start=(k == 0), stop=(k == KT - 1))
                nc.vector.tensor_copy(v_sb[:, j, 0:512], vps[:])
                vps2 = mmpool.tile([128, 8], F32, tag="mm")
                for k in range(KT):
                    nc.tensor.matmul(
                        vps2[:], xT_sb[:, k, jj:jj + 128],
                        wv_sb[:, k, 512:520],
                        start=(k == 0), stop=(k == KT - 1))
                nc.vector.tensor_copy(v_sb[:, j, 512:520], vps2[:])
                vones = v_sb[:, j, :].rearrange("p (h e) -> p h e", e=65)[:, :, 64]
                nc.vector.memset(vones, 1.0)

            def proj_groups(qc):
                return [lambda m=m: qk_group(qc, m) for m in range(8)] + \
                       [lambda j=j: v_group(j) for j in range(4 * qc, 4 * qc + 4)]

            def attention_pair(qc, hp, yraw_sb, den8_sb, fill=None):
                q0 = qc * 512
                hA, hB = 2 * hp, 2 * hp + 1
                avA = avpool.tile([65, 512], F32, tag="av", name=f"avA_{qc}_{hp}")
                avB = avpool.tile([65, 512], F32, tag="av", name=f"avB_{qc}_{hp}")
                # rounds: (j, psum offset, q offset, width) pairs; full-width
                # off-diagonal rounds first, then the two triangular diagonal
                # rounds with masked-out columns trimmed away
                rounds = []
                for jg in range(2 * qc):
                    rounds.append(([(2 * jg, 0, 0, 512), (2 * jg + 1, 512, 0, 512)],
                                   False))
                rounds.append(([(4 * qc, 0, 0, 512), (4 * qc + 1, 512, 128, 384)],
                               True))
                rounds.append(([(4 * qc + 2, 0, 256, 256), (4 * qc + 3, 256, 384, 128)],
                               True))
                for ri, (blocks, diag) in enumerate(rounds):
                    sA = spool.tile([128, 1024], F32, tag="s", name=f"sA_{qc}_{hp}_{ri}")
                    sB = spool.tile([128, 1024], F32, tag="s", name=f"sB_{qc}_{hp}_{ri}")
                    # the two j-blocks of one head's round target disjoint PE
                    # row groups (block 1 reads the partition-swapped copy), so
                    # they stream through the array concurrently
                    for j, off, qo, w in blocks:
                        for h, s_ps in ((hA, sA), (hB, sB)):
                            pb = (h % 2) * 64
                            nc.tensor.matmul(
                                s_ps[:, off:off + w],
                                qk_sb[pb:pb + 64, 4 + hp, j * 128:(j + 1) * 128],
                                qk_sb[pb:pb + 64, hp, q0 + qo:q0 + 512],
                                start=True, stop=True,
                                tile_position=(pb, 0))
                    wtot = blocks[-1][1] + blocks[-1][3]
                    for h, s_ps, av_ps in ((hA, sA, avA), (hB, sB, avB)):
                        e_sb = epool.tile([128, 1024], BF, tag="e")
                        nc.scalar.activation(e_sb[:, 0:wtot], s_ps[:, 0:wtot],
                                             AF.Exp, scale=0.125)
                        if diag:
                            moff = 0 if blocks[0][3] == 512 else 896
                            nc.vector.tensor_mul(
                                e_sb[:, 0:wtot], e_sb[:, 0:wtot],
                                mask_sb[:, moff:moff + wtot])
                        for j, off, qo, w in blocks:
                            nc.tensor.matmul(
                                av_ps[:, qo:512], v_sb[:, j, h * 65:h * 65 + 65],
                                e_sb[:, off:off + w],
                                start=(j == 0), stop=(j == 4 * qc + 3))
                    if fill is not None:
                        fill()
                # stash y and denominator; av psum slots free right away
                for h, av_ps in ((hA, avA), (hB, avB)):
                    with nc.allow_low_precision(reason="attention y bf16"):
                        nc.vector.tensor_copy(yraw_sb[:, h, :], av_ps[0:64, :])
                    p32 = (h % 4) * 32
                    nc.vector.tensor_copy(
                        den8_sb[p32:p32 + 1, h // 4, :], av_ps[64:65, :])

            def normalize_half(qc, half, yraw_sb, den8_sb):
                # heads 4*half .. 4*half+3 finished: reciprocal + scale them
                q0 = qc * 512
                r4_sb = nrmpool.tile([128, 512], F32, tag="r4", name=f"r4_{qc}_{half}")
                nc.vector.reciprocal_approx_fast(r4_sb[:], den8_sb[:, half, :])
                for h in range(4 * half, 4 * half + 4):
                    pb = (h % 2) * 64
                    p32 = (h % 4) * 32
                    r1_sb = nrmpool.tile([1, 512], BF, tag="r1")
                    with nc.allow_low_precision(reason="softmax denom bf16"):
                        nc.vector.tensor_copy(r1_sb[:], r4_sb[p32:p32 + 1, :])
                    # broadcast r across 64 partitions with a K=1 matmul into a
                    # spare slot of the av psum pool (idle at half boundaries)
                    bc_ps = avpool.tile([128, 512], F32, tag="av",
                                        name=f"bc_{qc}_{h}")[0:64, :]
                    nc.tensor.matmul(bc_ps, ones_sb[:], r1_sb[:], start=True, stop=True)
                    with nc.allow_low_precision(reason="attention y bf16"):
                        nc.vector.tensor_mul(
                            yT_sb[pb:pb + 64, h // 2, q0:q0 + 512],
                            yraw_sb[:, h, :], bc_ps)

            def outproj_group(qc, tt):
                t0 = (qc * 4 + tt) * 128
                osb = outpool.tile([128, 1024], BF, tag="osb")
                for n in range(2):
                    ops = mmpool.tile([128, 512], F32, tag="mm")
                    for cp in range(4):
                        nc.tensor.matmul(
                            ops[:], yT_sb[:, cp, t0:t0 + 128],
                            wp_sb[:, cp, n * 512:(n + 1) * 512],
                            start=(cp == 0), stop=(cp == 3))
                    with nc.allow_low_precision(reason="bf16 partial output"):
                        nc.vector.tensor_copy(osb[:, n * 512:(n + 1) * 512], ops[:])
                nc.sync.dma_start(out[t0:t0 + 128, :], osb[:])

            def outproj_groups(qc):
                return [lambda tt=tt: outproj_group(qc, tt) for tt in range(4)]

            def boot_qk_proj():
                # chunk-0 qT/kT projection with k as the OUTER loop: all 8
                # column-slot accumulators live at once (attention pools are
                # idle at kernel start, so all 8 PSUM banks are free) and the
                # first matmuls issue after only the first k-slice of DMA
                ps = []
                for m in range(8):
                    if m < 2:
                        ps.append(mmpool.tile([128, 512], F32, tag="mm",
                                              name=f"boot{m}"))
                    elif m < 6:
                        if m % 2 == 0:
                            st = spool.tile([128, 1024], F32, tag="s",
                                            name=f"boot{m}")
                        ps.append(st[:, (m % 2) * 512:(m % 2) * 512 + 512])
                    else:
                        ps.append(avpool.tile([128, 512], F32, tag="av",
                                              name=f"boot{m}"))
                for k in range(KT):
                    for m in range(8):
                        nc.tensor.matmul(
                            ps[m], wqk_sb[:, k, m * 128:(m + 1) * 128],
                            xT_sb[:, k, 0:512],
                            start=(k == 0), stop=(k == KT - 1))
                for m in range(8):
                    nc.vector.tensor_copy(qk_sb[:, m, 0:512], ps[m])

            # software pipeline: projection/output matmul groups are emitted as
            # fillers inside the (ScalarE-bound) attention stretches. Each
            # filler is tagged with the q chunk whose attention requires it;
            # they are force-drained before that attention is emitted.
            from collections import deque
            boot_qk_proj()
            for g in proj_groups(0)[8:]:
                g()
            fillers = deque((1, g) for g in proj_groups(1))
            for qc in range(NQC):
                due = [g for tag, g in fillers if tag <= qc]
                fillers = deque((tag, g) for tag, g in fillers if tag > qc)
                for g in due:
                    g()
                yraw_sb = nrmpool.tile([64, NHL, 512], BF, tag="yraw", name=f"yraw{qc}")
                den8_sb = nrmpool.tile([128, 2, 512], F32, tag="den8", name=f"den8{qc}")
                nc.vector.memset(den8_sb[:], 1.0)
                nfill = [4, 3, 3, 4][qc]
                for hp in range(4):
                    attention_pair(qc, hp, yraw_sb, den8_sb)
                    if hp == 1:
                        normalize_half(qc, 0, yraw_sb, den8_sb)
                    npop = nfill if hp < 3 else nfill - 2
                    for _ in range(npop):
                        if fillers:
                            fillers.popleft()[1]()
                # keep two filler groups to cover the reciprocal chain latency
                for _ in range(2):
                    if fillers:
                        fillers.popleft()[1]()
                normalize_half(qc, 1, yraw_sb, den8_sb)
                fillers.extend((NQC, g) for g in outproj_groups(qc))
                if qc + 2 < NQC:
                    fillers.extend((qc + 2, g) for g in proj_groups(qc + 2))
            while fillers:
                fillers.popleft()[1]()
    nc.compile()
    return nc


def _get_nc():
    if "nc" not in _CACHE:
        _CACHE["nc"] = _build()
    return _CACHE["nc"]


def _host_prep(x, W_attn, W_proj):
    """Shard + lay out per-core inputs. Returns list of 8 in_maps."""
    x = np.asarray(x, dtype=np.float32)
    W_attn = np.asarray(W_attn, dtype=np.float32)
    W_proj = np.asarray(W_proj, dtype=np.float32)

    # triangular mask prefix: mask[s, i] = 1.0 if s <= i else 0
    s_idx = np.arange(128)[:, None]
    q_idx = np.arange(512)[None, :]
    tri = (s_idx <= q_idx).astype(np.float32)
    mask = np.ascontiguousarray(np.concatenate(
        [tri[:, :512], tri[:, :384], tri[:, :256], tri[:, :128]], axis=1
    )).astype(Bb16)

    xT_b = [np.ascontiguousarray(x[b].T).astype(Bb16) for b in range(B)]
    in_maps = []
    for core in range(8):
        b, g = core // 2, core % 2
        c0 = g * 512
        wqk_g = np.concatenate(
            [W_attn[:, c0:c0 + 512], W_attn[:, C + c0:C + c0 + 512]], axis=1
        ).astype(Bb16)
        vbase = W_attn[:, 2 * C + c0:2 * C + c0 + 512]
        wv_g = np.zeros((C, 520), dtype=np.float32)
        for h in range(NHL):
            wv_g[:, h * 65:h * 65 + 64] = vbase[:, h * 64:(h + 1) * 64]
        wp_g = np.ascontiguousarray(W_proj[c0:c0 + 512, :]).astype(Bb16)
        in_maps.append({
            "xT": xT_b[b],
            "wqk": np.ascontiguousarray(wqk_g),
            "wv": wv_g.astype(Bb16),
            "wp": wp_g,
            "mask": mask,
        })
    return in_maps


def kernel(x, W_attn, W_proj):
    from concourse import bass_utils

    nc = _get_nc()
    in_maps = _host_prep(x, W_attn, W_proj)
    res = bass_utils.run_bass_kernel_spmd(nc, in_maps, core_ids=list(range(8)))
    outs = [res.results[c]["out"] for c in range(8)]
    full = np.empty((B, T, C), dtype=np.float32)
    for b in range(B):
        full[b] = outs[2 * b].astype(np.float32) + outs[2 * b + 1].astype(np.float32)
    return full


# revision 54
# speedup vs baseline: 1.2341x; 1.0414x over previous
"""Causal multi-head self-attention (B=4, T=2048, C=1024, H=16) on 8 TRN2 NeuronCores.

Sharding: core = b*2 + g  (b = batch 0..3, g = head-group 0..1 of 8 heads each).
Data parallel over batch; tensor parallel over heads (column-parallel W_attn,
row-parallel W_proj). Each core returns a partial (T, C) output; the host sums
the two partials per batch (the TP all-reduce happens in the unshard step).

Per-core device kernel (bf16 matmuls, f32 accumulation), per 512-wide q chunk:
  1. qT/kT projection with heads on partitions; head pairs share a 128-row tile
  2. v_aug projection in natural [t, c] layout with an all-ones column per head
     (the ones column turns the softmax denominator into row 64 of the y^T psum)
  3. attention in transposed [s, q] layout -- no transposes anywhere:
       S^T block = (kT block)^T @ qT chunk  (head-even rows 0:64 / head-odd rows
       64:128 of the PE array run concurrently: disjoint row groups)
       E = exp(S/8) on ScalarE (no max-subtraction: |scores|/8 < ~7)
       causal mask = precomputed multiplicative 0/1 tile on diagonal blocks
       y^T_aug accumulates v_aug^T @ E over s blocks in PSUM
     y/denominator are copied out of PSUM immediately (VectorE), denominators of
     all 8 heads take one batched reciprocal_approx_fast, the per-q reciprocal
     row is broadcast across partitions with a free-dim-step-0 SBUF->SBUF DMA,
     and one VectorE multiply normalizes.
  4. row-parallel output projection of the finished q chunk (overlaps the next
     chunk's attention).
"""

import numpy as np
import ml_dtypes

B, T, C, H = 4, 2048, 1024, 16
HS = C // H          # 64
NHL = 8              # local heads per core
KT = C // 128        # 8 contraction subtiles
NQC = T // 512       # 4 query chunks
NTB = T // 128       # 16 t-blocks
Bb16 = ml_dtypes.bfloat16

_CACHE = {}


def _build():
    import concourse.bass as bass
    import concourse.bacc as bacc
    import concourse.tile as tile
    import concourse.mybir as mybir

    BF = mybir.dt.bfloat16
    F32 = mybir.dt.float32
    AF = mybir.ActivationFunctionType

    nc = bacc.Bacc("TRN2", target_bir_lowering=False, debug=False, num_devices=8)
    xT = nc.dram_tensor("xT", [C, T], BF, kind="ExternalInput").ap()
    wqk = nc.dram_tensor("wqk", [C, 1024], BF, kind="ExternalInput").ap()
    wv = nc.dram_tensor("wv", [C, 520], BF, kind="ExternalInput").ap()
    wp = nc.dram_tensor("wp", [512, C], BF, kind="ExternalInput").ap()
    mask = nc.dram_tensor("mask", [128, 1280], BF, kind="ExternalInput").ap()
    out = nc.dram_tensor("out", [T, C], BF, kind="ExternalOutput").ap()

    with tile.TileContext(nc) as tc:
        with tc.tile_pool(name="persist", bufs=1) as persist, \
             tc.tile_pool(name="mm", bufs=2, space="PSUM") as mmpool, \
             tc.tile_pool(name="s", bufs=2, space="PSUM") as spool, \
             tc.tile_pool(name="av", bufs=2, space="PSUM") as avpool, \
             tc.tile_pool(name="e", bufs=4) as epool, \
             tc.tile_pool(name="nrm", bufs=2) as nrmpool, \
             tc.tile_pool(name="osb", bufs=3) as outpool:

            xT_sb = persist.tile([128, KT, T], BF, tag="xT")
            wqk_sb = persist.tile([128, KT, 1024], BF, tag="wqk")
            wv_sb = persist.tile([128, KT, 520], BF, tag="wv")
            wp_sb = persist.tile([128, 4, 1024], BF, tag="wp")
            mask_sb = persist.tile([128, 1280], BF, tag="mask")
            qk_sb = persist.tile([128, 8, T], BF, tag="qk")
            v_sb = persist.tile([128, NTB, 520], BF, tag="v")
            yT_sb = persist.tile([128, 4, T], BF, tag="yT")
            ones_sb = persist.tile([1, 64], BF, tag="ones")
            nc.vector.memset(ones_sb[:], 1.0)

            # load order: exactly what the first projection chunk needs, first
            for k in range(KT):
                nc.sync.dma_start(wqk_sb[:, k, :], wqk[k * 128:(k + 1) * 128, :])
                nc.sync.dma_start(xT_sb[:, k, 0:512], xT[k * 128:(k + 1) * 128, 0:512])
            for k in range(KT):
                nc.sync.dma_start(wv_sb[:, k, :], wv[k * 128:(k + 1) * 128, :])
            nc.sync.dma_start(mask_sb[:], mask[:])
            for k in range(KT):
                nc.sync.dma_start(xT_sb[:, k, 512:1024],
                                  xT[k * 128:(k + 1) * 128, 512:1024])
            for k in range(4):
                nc.sync.dma_start(wp_sb[:, k, :], wp[k * 128:(k + 1) * 128, :])
            for k in range(KT):
                nc.sync.dma_start(xT_sb[:, k, 1024:2048],
                                  xT[k * 128:(k + 1) * 128, 1024:2048])

            def qk_group(qc, m):
                q0 = qc * 512
                mm_ps = mmpool.tile([128, 512], F32, tag="mm")
                for k in range(KT):
                    nc.tensor.matmul(
                        mm_ps[:], wqk_sb[:, k, m * 128:(m + 1) * 128],
                        xT_sb[:, k, q0:q0 + 512],
                        start=(k == 0), stop=(k == KT - 1))
                nc.vector.tensor_copy(qk_sb[:, m, q0:q0 + 512], mm_ps[:])

            def v_group(j):
                jj = j * 128
                vps = mmpool.tile([128, 512], F32, tag="mm")
                for k in range(KT):
                    nc.tensor.matmul(
                        vps[:], xT_sb[:, k, jj:jj + 128],
                        wv_sb[:, k, 0:512],
                        start=(k == 0), stop=(k == KT - 1))
                nc.vector.tensor_copy(v_sb[:, j, 0:512], vps[:])
                vps2 = mmpool.tile([128, 8], F32, tag="mm")
                for k in range(KT):
                    nc.tensor.matmul(
                        vps2[:], xT_sb[:, k, jj:jj + 128],
                        wv_sb[:, k, 512:520],
                        start=(k == 0), stop=(k == KT - 1))
                nc.vector.tensor_copy(v_sb[:, j, 512:520], vps2[:])
                vones = v_sb[:, j, :].rearrange("p (h e) -> p h e", e=65)[:, :, 64]
                nc.vector.memset(vones, 1.0)

            def proj_groups(qc):
                return [lambda m=m: qk_group(qc, m) for m in range(8)] + \
                       [lambda j=j: v_group(j) for j in range(4 * qc, 4 * qc + 4)]

            def attention_pair(qc, hp, yraw_sb, den8_sb, fill=None):
                q0 = qc * 512
                hA, hB = 2 * hp, 2 * hp + 1
                avA = avpool.tile([65, 512], F32, tag="av", name=f"avA_{qc}_{hp}")
                avB = avpool.tile([65, 512], F32, tag="av", name=f"avB_{qc}_{hp}")
                # rounds: (j, psum offset, q offset, width) pairs; full-width
                # off-diagonal rounds first, then the two triangular diagonal
                # rounds with masked-out columns trimmed away
                rounds = []
                for jg in range(2 * qc):
                    rounds.append(([(2 * jg, 0, 0, 512), (2 * jg + 1, 512, 0, 512)],
                                   False))
                rounds.append(([(4 * qc, 0, 0, 512), (4 * qc + 1, 512, 128, 384)],
                               True))
                rounds.append(([(4 * qc + 2, 0, 256, 256), (4 * qc + 3, 256, 384, 128)],
                               True))
                for ri, (blocks, diag) in enumerate(rounds):
                    sA = spool.tile([128, 1024], F32, tag="s", name=f"sA_{qc}_{hp}_{ri}")
                    sB = spool.tile([128, 1024], F32, tag="s", name=f"sB_{qc}_{hp}_{ri}")
                    # the two j-blocks of one head's round target disjoint PE
                    # row groups (block 1 reads the partition-swapped copy), so
                    # they stream through the array concurrently
                    for j, off, qo, w in blocks:
                        for h, s_ps in ((hA, sA), (hB, sB)):
                            pb = (h % 2) * 64
                            nc.tensor.matmul(
                                s_ps[:, off:off + w],
                                qk_sb[pb:pb + 64, 4 + hp, j * 128:(j + 1) * 128],
                                qk_sb[pb:pb + 64, hp, q0 + qo:q0 + 512],
                                start=True, stop=True,
                                tile_position=(pb, 0))
                    wtot = blocks[-1][1] + blocks[-1][3]
                    for h, s_ps, av_ps in ((hA, sA, avA), (hB, sB, avB)):
                        e_sb = epool.tile([128, 1024], BF, tag="e")
                        nc.scalar.activation(e_sb[:, 0:wtot], s_ps[:, 0:wtot],
                                             AF.Exp, scale=0.125)
                        if diag:
                            moff = 0 if blocks[0][3] == 512 else 896
                            nc.vector.tensor_mul(
                                e_sb[:, 0:wtot], e_sb[:, 0:wtot],
                                mask_sb[:, moff:moff + wtot])
                        for j, off, qo, w in blocks:
                            nc.tensor.matmul(
                                av_ps[:, qo:512], v_sb[:, j, h * 65:h * 65 + 65],
                                e_sb[:, off:off + w],
                                start=(j == 0), stop=(j == 4 * qc + 3))
                    if fill is not None:
                        fill()
                # stash y and denominator; av psum slots free right away
                for h, av_ps in ((hA, avA), (hB, avB)):
                    with nc.allow_low_precision(reason="attention y bf16"):
                        nc.vector.tensor_copy(yraw_sb[:, h, :], av_ps[0:64, :])
                    p32 = (h % 4) * 32
                    nc.vector.tensor_copy(
                        den8_sb[p32:p32 + 1, h // 4, :], av_ps[64:65, :])

            def normalize_half(qc, half, yraw_sb, den8_sb):
                # heads 4*half .. 4*half+3 finished: reciprocal + scale them
                q0 = qc * 512
                r4_sb = nrmpool.tile([128, 512], F32, tag="r4", name=f"r4_{qc}_{half}")
                nc.vector.reciprocal_approx_fast(r4_sb[:], den8_sb[:, half, :])
                for h in range(4 * half, 4 * half + 4):
                    pb = (h % 2) * 64
                    p32 = (h % 4) * 32
                    r1_sb = nrmpool.tile([1, 512], BF, tag="r1")
                    with nc.allow_low_precision(reason="softmax denom bf16"):
                        nc.vector.tensor_copy(r1_sb[:], r4_sb[p32:p32 + 1, :])
                    # broadcast r across 64 partitions with a K=1 matmul into a
                    # spare slot of the av psum pool (idle at half boundaries)
                    bc_ps = avpool.tile([128, 512], F32, tag="av",
                                        name=f"bc_{qc}_{h}")[0:64, :]
                    nc.tensor.matmul(bc_ps, ones_sb[:], r1_sb[:], start=True, stop=True)
                    with nc.allow_low_precision(reason="attention y bf16"):
                        nc.vector.tensor_mul(
                            yT_sb[pb:pb + 64, h // 2, q0:q0 + 512],
                            yraw_sb[:, h, :], bc_ps)

            def outproj_group(qc, tt):
                t0 = (qc * 4 + tt) * 128
                osb = outpool.tile([128, 1024], BF, tag="osb")
                for n in range(2):
                    ops = mmpool.tile([128, 512], F32, tag="mm")
                    for cp in range(4):
                        nc.tensor.matmul(
                            ops[:], yT_sb[:, cp, t0:t0 + 128],
                            wp_sb[:, cp, n * 512:(n + 1) * 512],
                            start=(cp == 0), stop=(cp == 3))
                    with nc.allow_low_precision(reason="bf16 partial output"):
                        nc.vector.tensor_copy(osb[:, n * 512:(n + 1) * 512], ops[:])
                nc.sync.dma_start(out[t0:t0 + 128, :], osb[:])

            def outproj_groups(qc):
                return [lambda tt=tt: outproj_group(qc, tt) for tt in range(4)]

            def boot_qk_proj():
                # chunk-0 qT/kT projection with k as the OUTER loop: all 8
                # column-slot accumulators live at once (attention pools are
                # idle at kernel start, so all 8 PSUM banks are free) and the
                # first matmuls issue after only the first k-slice of DMA
                ps = []
                for m in range(8):
                    if m < 2:
                        ps.append(mmpool.tile([128, 512], F32, tag="mm",
                                              name=f"boot{m}"))
                    elif m < 6:
                        if m % 2 == 0:
                            st = spool.tile([128, 1024], F32, tag="s",
                                            name=f"boot{m}")
                        ps.append(st[:, (m % 2) * 512:(m % 2) * 512 + 512])
                    else:
                        ps.append(avpool.tile([128, 512], F32, tag="av",
                                              name=f"boot{m}"))
                for k in range(KT):
                    for m in range(8):
                        nc.tensor.matmul(
                            ps[m], wqk_sb[:, k, m * 128:(m + 1) * 128],
                            xT_sb[:, k, 0:512],
                            start=(k == 0), stop=(k == KT - 1))
                for m in range(8):
                    nc.vector.tensor_copy(qk_sb[:, m, 0:512], ps[m])

            # software pipeline: projection/output matmul groups are emitted as
            # fillers inside the (ScalarE-bound) attention stretches. Each
            # filler is tagged with the q chunk whose attention requires it;
            # they are force-drained before that attention is emitted.
            from collections import deque
            boot_qk_proj()
            for g in proj_groups(0)[8:]:
                g()
            fillers = deque((1, g) for g in proj_groups(1))
            for qc in range(NQC):
                due = [g for tag, g in fillers if tag <= qc]
                fillers = deque((tag, g) for tag, g in fillers if tag > qc)
                for g in due:
                    g()
                yraw_sb = nrmpool.tile([64, NHL, 512], BF, tag="yraw", name=f"yraw{qc}")
                den8_sb = nrmpool.tile([128, 2, 512], F32, tag="den8", name=f"den8{qc}")
                nc.vector.memset(den8_sb[:], 1.0)
                nfill = [4, 3, 3, 0][qc]
                for hp in range(4):
                    attention_pair(qc, hp, yraw_sb, den8_sb)
                    if hp == 1:
                        normalize_half(qc, 0, yraw_sb, den8_sb)
                    npop = nfill if hp < 3 else max(0, nfill - 2)
                    for _ in range(npop):
                        if fillers:
                            fillers.popleft()[1]()
                # keep filler groups back to cover the reciprocal chain latency
                for _ in range(2 if qc < 3 else 4):
                    if fillers:
                        fillers.popleft()[1]()
                normalize_half(qc, 1, yraw_sb, den8_sb)
                fillers.extend((NQC, g) for g in outproj_groups(qc))
                if qc + 2 < NQC:
                    fillers.extend((qc + 2, g) for g in proj_groups(qc + 2))
            while fillers:
                fillers.popleft()[1]()
    nc.compile()
    return nc


def _get_nc():
    if "nc" not in _CACHE:
        _CACHE["nc"] = _build()
    return _CACHE["nc"]


def _host_prep(x, W_attn, W_proj):
    """Shard + lay out per-core inputs. Returns list of 8 in_maps."""
    x = np.asarray(x, dtype=np.float32)
    W_attn = np.asarray(W_attn, dtype=np.float32)
    W_proj = np.asarray(W_proj, dtype=np.float32)

    # triangular mask prefix: mask[s, i] = 1.0 if s <= i else 0
    s_idx = np.arange(128)[:, None]
    q_idx = np.arange(512)[None, :]
    tri = (s_idx <= q_idx).astype(np.float32)
    mask = np.ascontiguousarray(np.concatenate(
        [tri[:, :512], tri[:, :384], tri[:, :256], tri[:, :128]], axis=1
    )).astype(Bb16)

    xT_b = [np.ascontiguousarray(x[b].T).astype(Bb16) for b in range(B)]
    in_maps = []
    for core in range(8):
        b, g = core // 2, core % 2
        c0 = g * 512
        wqk_g = np.concatenate(
            [W_attn[:, c0:c0 + 512], W_attn[:, C + c0:C + c0 + 512]], axis=1
        ).astype(Bb16)
        vbase = W_attn[:, 2 * C + c0:2 * C + c0 + 512]
        wv_g = np.zeros((C, 520), dtype=np.float32)
        for h in range(NHL):
            wv_g[:, h * 65:h * 65 + 64] = vbase[:, h * 64:(h + 1) * 64]
        wp_g = np.ascontiguousarray(W_proj[c0:c0 + 512, :]).astype(Bb16)
        in_maps.append({
            "xT": xT_b[b],
            "wqk": np.ascontiguousarray(wqk_g),
            "wv": wv_g.astype(Bb16),
            "wp": wp_g,
            "mask": mask,
        })
    return in_maps


def kernel(x, W_attn, W_proj):
    from concourse import bass_utils

    nc = _get_nc()
    in_maps = _host_prep(x, W_attn, W_proj)
    res = bass_utils.run_bass_kernel_spmd(nc, in_maps, core_ids=list(range(8)))
    outs = [res.results[c]["out"] for c in range(8)]
    full = np.empty((B, T, C), dtype=np.float32)
    for b in range(B):
        full[b] = outs[2 * b].astype(np.float32) + outs[2 * b + 1].astype(np.float32)
    return full
